# revision 1
# baseline (speedup 1.0000x reference)
import numpy as np
import concourse.bass as bass
import concourse.bacc as bacc
import concourse.mybir as mybir
from concourse.tile import TileContext
from concourse.bass_utils import run_bass_kernel_spmd

B, HID = 4096, 512
NR, NB = 32, 8
T = 32
OPB, AB, LB, NOPS = 2, 5, 5, 4
G = 8
NCORES = 8
BC = B // NCORES          # 512 batch rows per core
P = 128
NBLK = BC // P            # 4 blocks per core
COLS = NR * NB + T * OPB + 3 * T * AB + LB   # 805

# column offsets inside the concatenated weight matrix
OFF_R, OFF_OP, OFF_D, OFF_1, OFF_2, OFF_L = 0, 256, 320, 480, 640, 800

f32 = mybir.dt.float32
f16 = mybir.dt.float16
AX = mybir.AxisListType
OP = mybir.AluOpType
AF = mybir.ActivationFunctionType

_STATE = {}


def _build(repeat=1):
    nc = bacc.Bacc("TRN2", target_bir_lowering=False, debug=False,
                   num_devices=NCORES)
    z_d = nc.declare_dram_parameter("z", [BC, HID], f32, isOutput=False)
    wc_d = nc.declare_dram_parameter("wcat", [HID, COLS], f32, isOutput=False)
    pw_d = nc.declare_dram_parameter("pw", [P, COLS], f16, isOutput=False)
    ri_d = nc.declare_dram_parameter("ri", [P, NR], f16, isOutput=False)
    rn_d = nc.declare_dram_parameter("rin", [P, NR], f32, isOutput=False)
    rp_d = nc.declare_dram_parameter("rip", [P, NR], f16, isOutput=False)
    ki_d = nc.declare_dram_parameter("ki", [P, NOPS], f16, isOutput=False)
    tg_d = nc.declare_dram_parameter("tg", [P, T], f32, isOutput=False)
    id_d = nc.declare_dram_parameter("ident", [P, P], f32, isOutput=False)
    idh_d = nc.declare_dram_parameter("identh", [P, P], f16, isOutput=False)
    w2_d = nc.declare_dram_parameter("w2tb", [NR + 1, HID], f16, isOutput=False)
    lg_d = nc.declare_dram_parameter("lng", [P, HID], f16, isOutput=False)
    lb_d = nc.declare_dram_parameter("lnb", [P, HID], f16, isOutput=False)
    out_d = nc.declare_dram_parameter("out", [BC, G * HID], f16, isOutput=True)

    delta = np.linspace(-1.0, 1.0, G).astype(np.float32)

    with TileContext(nc) as tc:
        with tc.tile_pool(name="const", bufs=1) as cp, \
             tc.tile_pool(name="work", bufs=1) as wp, \
             tc.tile_pool(name="pp", bufs=2) as pp, \
             tc.tile_pool(name="scr", bufs=2) as sp, \
             tc.psum_pool(name="pst", bufs=2) as pt, \
             tc.psum_pool(name="psl", bufs=1) as pl, \
             tc.psum_pool(name="psh", bufs=2) as ph:
            # ---- constants ----
            wc = cp.tile([P, 4, COLS], f32)
            nc.gpsimd.dma_start(wc[:], wc_d[:].rearrange("(k p) c -> p k c", k=4))
            pwr = cp.tile([P, COLS], f16)
            nc.gpsimd.dma_start(pwr[:], pw_d[:])
            rin = cp.tile([P, NR], f32)
            nc.gpsimd.dma_start(rin[:], rn_d[:])
            rinn = cp.tile([P, NR], f16)
            nc.gpsimd.dma_start(rinn[:], rp_d[:])
            tgr = cp.tile([P, T], f32)
            nc.gpsimd.dma_start(tgr[:], tg_d[:])
            ident = cp.tile([P, P], f32)
            nc.gpsimd.dma_start(ident[:], id_d[:])
            identh = cp.tile([P, P], f16)
            nc.gpsimd.dma_start(identh[:], idh_d[:])
            w2tb = cp.tile([NR + 1, HID], f16)
            nc.gpsimd.dma_start(w2tb[:], w2_d[:])
            lngr = cp.tile([P, HID], f16)
            nc.gpsimd.dma_start(lngr[:], lg_d[:])
            lnbr = cp.tile([P, HID], f16)
            nc.gpsimd.dma_start(lnbr[:], lb_d[:])
            bt = cp.tile([P, G + 4], f32)
            for i in range(G):
                nc.vector.memset(bt[:, i:i + 1], float(delta[i]))
            nc.vector.memset(bt[:, G:G + 1], 1e-5)
            nc.vector.memset(bt[:, G + 1:G + 2], 0.5)
            nc.vector.memset(bt[:, G + 2:G + 3], float(NR) - 0.5)
            nc.vector.memset(bt[:, G + 3:G + 4], float(NOPS) - 0.5)

            pwb = pwr[:]
            lngb = lngr[:]
            lnbb = lnbr[:]
            tgb = tgr[:].unsqueeze(1).broadcast_to([P, G, T])
            TH = T // 2
            tt = nc.vector.tensor_tensor

            def build_ndc_half(st, nb, ndc, cRM):
                # ndc[s,g,th,r] = cRM[s,g,st+th] * nb[2,g,st+th,r]
                nc.vector.tensor_tensor(
                    ndc[:, 0, :, :, :], nb[:, 2, :, st:st + TH, :],
                    cRM[:, 0, :, st:st + TH].unsqueeze(3)
                    .broadcast_to([P, G, TH, NR]), OP.mult)
                nc.gpsimd.tensor_tensor(
                    ndc[:, 1, :, :, :], nb[:, 2, :, st:st + TH, :],
                    cRM[:, 1, :, st:st + TH].unsqueeze(3)
                    .broadcast_to([P, G, TH, NR]), OP.mult)

            def front(blk):
                r0, r1 = blk * P, (blk + 1) * P
                # logits = z_blk @ W_cat
                zb = wp.tile([P, HID], f32)
                nc.gpsimd.dma_start(zb[:], z_d[r0:r1, :])
                zt = wp.tile([P, 4, P], f32)
                for k in range(4):
                    tp = pt.tile([P, P], f32)
                    nc.tensor.transpose(tp[:], zb[:, k * P:(k + 1) * P],
                                        ident[:])
                    nc.scalar.activation(zt[:, k, :], tp[:], AF.Copy)
                l1 = pl.tile([P, 512], f32)
                l2 = pl.tile([P, COLS - 512], f32)
                for k in range(4):
                    nc.tensor.matmul(l1[:], zt[:, k, :], wc[:, k, 0:512],
                                     start=(k == 0), stop=(k == 3))
                for k in range(4):
                    nc.tensor.matmul(l2[:], zt[:, k, :], wc[:, k, 512:COLS],
                                     start=(k == 0), stop=(k == 3))
                lg = wp.tile([P, COLS], f32)
                nc.scalar.activation(lg[:, 0:512], l1[:], AF.Copy)
                nc.scalar.activation(lg[:, 512:COLS], l2[:], AF.Copy)

                # per-candidate sigmoid decode -> decimals
                dvals = wp.tile([P, 3, G, T], f32)   # a-order [s1,s2,dst]
                opd = wp.tile([P, G, T], f32)
                plen = wp.tile([P, G], f32)
                decR = wp.tile([P, G, NR], f32)
                for g in range(G):
                    sigp = sp.tile([P, COLS], f16)
                    nc.scalar.activation(sigp[:], lg[:], AF.Sigmoid,
                                         bias=bt[:, g:g + 1])
                    nc.vector.tensor_tensor(sigp[:], sigp[:], pwb, OP.mult)
                    nc.vector.tensor_reduce(
                        decR[:, g, :],
                        sigp[:, OFF_R:OFF_OP].rearrange("p (r b) -> p r b",
                                                        r=NR),
                        AX.X, OP.add)
                    nc.vector.tensor_reduce(
                        opd[:, g, :],
                        sigp[:, OFF_OP:OFF_D].rearrange("p (t b) -> p t b",
                                                        t=T),
                        AX.X, OP.add)
                    nc.vector.tensor_reduce(
                        dvals[:, :, g, :],
                        sigp[:, OFF_D:OFF_L].rearrange("p (a t b) -> p a t b",
                                                       a=3, t=T),
                        AX.X, OP.add)
                    nc.vector.tensor_reduce(plen[:, g:g + 1],
                                            sigp[:, OFF_L:COLS], AX.X, OP.add)

                # register init -> fp16 state (ping-pong)
                S = pp.tile([P, 2, G, NR], f16)
                nc.vector.tensor_scalar_mul(S[:, 0, :, :], decR[:], 1.0)
                nc.vector.memset(S[:, 1, :, :], 0.0)

                # Z denominators via erf closed form
                Zb = wp.tile([P, 3, G, T], f16)
                iZ = pp.tile([P, 3, G, T], f16)
                ze2 = wp.tile([P, 3, G, T], f16)
                nc.scalar.activation(Zb[:], dvals[:], AF.Erf,
                                     bias=bt[:, G + 1:G + 2])
                nc.scalar.activation(ze2[:], dvals[:], AF.Erf,
                                     bias=bt[:, G + 2:G + 3], scale=-1.0)
                tt(Zb[:], Zb[:], ze2[:], OP.add)
                with nc.allow_low_precision(reason="fp16 softmax denom"):
                    nc.vector.reciprocal(iZ[:], Zb[:])
                Zop = wp.tile([P, G, T], f16)
                iZop = wp.tile([P, G, T], f16)
                zo2 = wp.tile([P, G, T], f16)
                nc.scalar.activation(Zop[:], opd[:], AF.Erf,
                                     bias=bt[:, G + 1:G + 2])
                nc.scalar.activation(zo2[:], opd[:], AF.Erf,
                                     bias=bt[:, G + 3:G + 4], scale=-1.0)
                tt(Zop[:], Zop[:], zo2[:], OP.add)
                with nc.allow_low_precision(reason="fp16 softmax denom"):
                    nc.vector.reciprocal(iZop[:], Zop[:])

                # soft halting mask
                actx = wp.tile([P, G, T], f16)
                nc.vector.scalar_tensor_tensor(
                    actx[:], plen[:].unsqueeze(2).broadcast_to([P, G, T]),
                    1.0, tgb, OP.mult, OP.subtract)
                nc.scalar.activation(actx[:], actx[:], AF.Sigmoid)

                # op softmax numerators: obt layed out [k, g, t] with a
                # bulk (opd - k) pass; strided activation outputs are slow
                # on HW, so subtract first, then one contiguous DErf.
                obx = wp.tile([P, G, T, NOPS], f16)
                opdf = opd[:].rearrange("p g t -> p (g t)")
                nc.vector.scalar_tensor_tensor(
                    obx[:].rearrange("p g t k -> p (g t) k"),
                    opdf.unsqueeze(2).broadcast_to([P, G * T, NOPS]), 1.0,
                    rin[:, 0:NOPS].unsqueeze(1)
                    .broadcast_to([P, G * T, NOPS]), OP.mult, OP.add)
                nc.scalar.activation(obx[:], obx[:], AF.Derivative_Erf)

                # address softmax numerators: nb = DErf(d - r), bulk diff
                nb = pp.tile([P, 3, G, T, NR], f16)
                nbv = nb[:].rearrange("p a g t r -> p (a g t) r")
                dvf = dvals[:].rearrange("p a g t -> p (a g t)")
                dvrep = wp.tile([P, 3 * G * T, 8], f16)
                nc.vector.tensor_scalar_mul(
                    dvrep[:], dvf.unsqueeze(2)
                    .broadcast_to([P, 3 * G * T, 8]), 1.0)
                nc.vector.tensor_tensor(
                    nbv[:].rearrange("p x (rr r) -> p x rr r", rr=4),
                    dvrep[:].unsqueeze(2).broadcast_to([P, 3 * G * T, 4, 8]),
                    rinn[:].rearrange("p (rr r) -> p rr r", rr=4)
                    .unsqueeze(1).broadcast_to([P, 3 * G * T, 4, 8]),
                    OP.subtract)
                nc.scalar.activation(nbv[:], nbv[:], AF.Derivative_Erf)

                # fold denominators into per-step coefficients
                # coefT kinds [A,C,B,D] pair with vbuf [v1n,lvn,v2n,dvn]
                coefT = pp.tile([P, 4, G, T], f16)
                cRM = pp.tile([P, 2, G, T], f16)
                iZ1 = iZ[:, 0, :, :]
                iZ2 = iZ[:, 1, :, :]
                iZd = iZ[:, 2, :, :]
                slotA = coefT[:, 0, :, :]
                slotC = coefT[:, 1, :, :]
                slotB = coefT[:, 2, :, :]
                slotD = coefT[:, 3, :, :]
                slot_cR = cRM[:, 0, :, :]
                slot_cM = cRM[:, 1, :, :]
                ob0 = obx[:, :, :, 0]
                ob1 = obx[:, :, :, 1]
                ob2 = obx[:, :, :, 2]
                ob3 = obx[:, :, :, 3]
                t1 = sp.tile([P, G, T], f16)
                t2 = sp.tile([P, G, T], f16)
                tt(t1[:], ob0, ob1, OP.add)
                tt(t1[:], t1[:], iZop[:], OP.mult)
                tt(slotA, t1[:], iZ1, OP.mult)
                tt(t1[:], ob0, ob1, OP.subtract)
                tt(t1[:], t1[:], iZop[:], OP.mult)
                tt(slotB, t1[:], iZ2, OP.mult)
                tt(t1[:], ob2, iZop[:], OP.mult)
                tt(slotC, t1[:], iZ1, OP.mult)
                tt(t2[:], ob3, iZop[:], OP.mult)
                tt(slotD, t2[:], iZd, OP.mult)
                tt(slot_cM, slotD, actx[:], OP.mult)
                nc.vector.tensor_scalar(t2[:], t2[:], -1.0, 1.0,
                                        OP.mult, OP.add)
                tt(t2[:], t2[:], iZd, OP.mult)
                tt(slot_cR, t2[:], actx[:], OP.mult)

                # first half of the gate tensor
                ndc = pp.tile([P, 2, G, TH, NR], f16)
                build_ndc_half(0, nb, ndc, cRM)
                return dict(S=S, iZ=iZ, nb=nb, coefT=coefT, cRM=cRM, ndc=ndc)

            def scan_ln(blk, tl):
                r0, r1 = blk * P, (blk + 1) * P
                S, iZ, nb = tl["S"], tl["iZ"], tl["nb"]
                coefT, cRM, ndc = tl["coefT"], tl["cRM"], tl["ndc"]
                q4 = wp.tile([P, 4, G, NR], f16)
                vbuf = wp.tile([P, 4, G], f32)    # [v1n, lvn, v2n, dvn]
                resP = wp.tile([P, 4, G], f32)
                targ = wp.tile([P, 2, G], f32)    # [res, v1]
                for t in range(T):
                    if t == TH:
                        build_ndc_half(TH, nb, ndc, cRM)
                    tt(q4[:, 0:2, :, :], S[:],
                       nb[:, 0, :, t, :].unsqueeze(1)
                       .broadcast_to([P, 2, G, NR]), OP.mult)
                    tt(q4[:, 2:4, :, :],
                       S[:, 0:1, :, :].broadcast_to([P, 2, G, NR]),
                       nb[:, 1:3, :, t, :], OP.mult)
                    nc.vector.tensor_reduce(vbuf[:], q4[:], AX.X, OP.add)
                    tt(resP[:], vbuf[:], coefT[:, :, :, t], OP.mult)
                    nc.vector.tensor_reduce(targ[:, 0, :],
                                            resP[:].transpose([0, 2, 1]),
                                            AX.X, OP.add)
                    tt(targ[:, 1, :], vbuf[:, 0, :], iZ[:, 0, :, t], OP.mult)
                    # u = S - targ ; v = ndc_t * u ; S -= v   (reuse q4)
                    tt(q4[:, 0:2, :, :], S[:],
                       targ[:].unsqueeze(3).broadcast_to([P, 2, G, NR]),
                       OP.subtract)
                    tt(q4[:, 2:4, :, :], q4[:, 0:2, :, :],
                       ndc[:, :, :, t % TH, :], OP.mult)
                    tt(S[:], S[:], q4[:, 2:4, :, :], OP.subtract)

                # register2hidden + LayerNorm per candidate
                hsumG = wp.tile([P, G], f32)
                vsumG = wp.tile([P, G], f32)
                negmuG = wp.tile([P, G], f32)
                stdG = wp.tile([P, G], f32)
                rstdG = wp.tile([P, G], f32)
                for g in range(G):
                    rp = ph.tile([NR, P], f16)
                    nc.tensor.transpose(rp[:], S[:, 0, g, :], identh[:])
                    rft = sp.tile([NR + 1, P], f16)
                    nc.scalar.activation(rft[0:NR, :], rp[:], AF.Copy)
                    nc.vector.memset(rft[NR:NR + 1, :], 1.0)
                    hp = ph.tile([P, HID], f32)
                    nc.tensor.matmul(hp[:], rft[:], w2tb[:], start=True,
                                     stop=True)
                    h = sp.tile([P, HID], f32)
                    nc.scalar.activation(h[:], hp[:], AF.Copy,
                                         accum_out=hsumG[:, g:g + 1])
                    nc.vector.tensor_scalar_mul(negmuG[:, g:g + 1],
                                                hsumG[:, g:g + 1], -1.0 / HID)
                    hc = sp.tile([P, HID], f16)
                    nc.vector.tensor_scalar_add(hc[:], h[:],
                                                negmuG[:, g:g + 1])
                    hsc = sp.tile([P, HID], f16)
                    nc.scalar.activation(hsc[:], hc[:], AF.Square,
                                         accum_out=vsumG[:, g:g + 1])
                    nc.scalar.activation(stdG[:, g:g + 1], vsumG[:, g:g + 1],
                                         AF.Sqrt, bias=bt[:, G:G + 1],
                                         scale=1.0 / HID)
                    nc.vector.reciprocal(rstdG[:, g:g + 1], stdG[:, g:g + 1])
                    nc.vector.tensor_scalar_mul(hsc[:], hc[:],
                                                rstdG[:, g:g + 1])
                    ot = sp.tile([P, HID], f16)
                    nc.vector.tensor_tensor(ot[:], hsc[:], lngb, OP.mult)
                    nc.vector.tensor_tensor(hsc[:], ot[:], lnbb, OP.add)
                    nc.gpsimd.dma_start(
                        out_d[r0:r1, g * HID:(g + 1) * HID], hsc[:])

            # software pipeline: front(k+1) issued before scan(k)
            NB_TOT = NBLK * repeat
            tiles = front(0)
            for k in range(NB_TOT):
                nxt = front((k + 1) % NBLK) if k + 1 < NB_TOT else None
                scan_ln(k % NBLK, tiles)
                tiles = nxt

    nc.compile()
    return nc


def _get_nc(repeat=1):
    key = f"nc{repeat}"
    if key not in _STATE:
        _STATE[key] = _build(repeat)
    return _STATE[key]


def _make_consts(inputs):
    f = lambda a: np.ascontiguousarray(np.asarray(a), dtype=np.float32)
    wcat = np.concatenate([f(inputs["W_R"]), f(inputs["W_op"]),
                           f(inputs["W_src1"]), f(inputs["W_src2"]),
                           f(inputs["W_dst"]), f(inputs["W_len"])], axis=1)
    pw8 = (2.0 ** np.arange(NB)).astype(np.float32)
    pw2 = (2.0 ** np.arange(OPB)).astype(np.float32)
    pw5 = (2.0 ** np.arange(AB)).astype(np.float32)
    pw = np.concatenate([np.tile(pw8, NR), np.tile(pw2, T),
                         np.tile(pw5, T), np.tile(pw5, T), np.tile(pw5, T),
                         pw5]).astype(np.float32)
    w2tb = np.vstack([f(inputs["W_r2h"]).T, f(inputs["b_r2h"])[None]])
    rep = lambda row: np.ascontiguousarray(np.tile(row[None], (P, 1)))
    return {
        "wcat": np.ascontiguousarray(wcat),
        "pw": rep(pw).astype(np.float16),
        "ri": rep(np.arange(NR, dtype=np.float32)).astype(np.float16),
        "rin": rep(-np.arange(NR, dtype=np.float32)),
        "rip": rep(np.arange(NR, dtype=np.float32)).astype(np.float16),
        "ki": rep(np.arange(NOPS, dtype=np.float32)).astype(np.float16),
        "tg": rep(np.arange(T, dtype=np.float32) + 0.5),
        "ident": np.eye(P, dtype=np.float32),
        "identh": np.eye(P, dtype=np.float16),
        "w2tb": np.ascontiguousarray(w2tb).astype(np.float16),
        "lng": rep(f(inputs["ln_g"])).astype(np.float16),
        "lnb": rep(f(inputs["ln_b"])).astype(np.float16),
    }


def kernel(**inputs) -> np.ndarray:
    nc = _get_nc()
    z = np.ascontiguousarray(np.asarray(inputs["z_hidden"]), dtype=np.float32)
    consts = _make_consts(inputs)
    in_maps = [dict(z=np.ascontiguousarray(z[c * BC:(c + 1) * BC]), **consts)
               for c in range(NCORES)]
    res = run_bass_kernel_spmd(nc, in_maps, list(range(NCORES)))
    out = np.concatenate(
        [np.asarray(res.results[c]["out"]) for c in range(NCORES)], axis=0)
    return out.astype(np.float32).reshape(B, G, HID)

